# revision 1
# baseline (speedup 1.0000x reference)
"""Trainium2 Bass kernel for the HGCA contrastive loss (nn_HGCA_10857677324785).

loss = mean over i of 0.5*(l1_i + l2_i) where
  h1 = elu(z1@W1+b1)@W2+b2 ; h2 likewise ; an, bn = l2-normalized rows
  l1_i = -log( exp(an_i.bn_i/tau) / (sum_j exp(an_i.an_j/tau)
               + sum_j exp(an_i.bn_j/tau) - e^{1/tau}) )
  l2_i symmetric with row sums of exp(bn@bn.T) and exp(bn@an.T).

Distribution: rows sharded over 8 cores. Host rolls z1/z2 per core so each
core's row block sits at local rows [0,2048). Each core computes the full
normalized projections (cheap, O(N D^2)), then its row-block of the three
N x N similarity matrices flash-style: exp row sums on ACT (fused accum),
plus per-column partial sums of exp(an@bn.T) (for l2's "between" term, which
equals column sums of the l1 "between" matrix). Host assembles the scalar
loss from O(N) partial sums.
"""

import re

import ml_dtypes
import numpy as np

import concourse.bass as bass
import concourse.tile as tile
from concourse import mybir
from concourse.bass_utils import run_bass_kernel_spmd
from concourse.masks import make_identity
from concourse.vector_clock import ScopedClock, VectorClock

N = 16384
D = 128
NCORES = 8
R = N // NCORES  # 2048 rows per core
INV_TAU = 2.0  # 1/0.5
F32 = mybir.dt.float32
BF16 = mybir.dt.bfloat16
AF = mybir.ActivationFunctionType
OP = mybir.AluOpType

# This walrus build supports at most 2 sync waits per instruction; Tile's sem
# assignment freely emits 3-11. Post-pass: hoist excess waits onto injected
# same-engine EventSemaphore fillers (engine queues are FIFO, so waits on an
# earlier filler happen-before the original instruction executes).

_MAX_WAITS = 1


def _split_waits(nc):
    for fn in nc.m.functions:
        for bb in fn.blocks:
            insts = list(bb.instructions)
            out = []
            changed = False
            for inst in insts:
                si = inst.sync_info
                w = list(si.on_wait) if si and si.on_wait else []
                if len(w) > _MAX_WAITS:
                    changed = True
                    extra, keep = w[:-_MAX_WAITS], w[-_MAX_WAITS:]
                    for i in range(0, len(extra), _MAX_WAITS):
                        f = mybir.InstEventSemaphore(
                            name=f"{inst.name}_wsplit{i}",
                            engine=inst.engine,
                            ins=[],
                            outs=[],
                            sync_info=mybir.SyncInfo(
                                on_wait=extra[i : i + _MAX_WAITS], on_update=[]
                            ),
                        )
                        out.append(f)
                    inst.sync_info = mybir.SyncInfo(
                        on_wait=keep,
                        on_update=list(si.on_update) if si.on_update else [],
                    )
                out.append(inst)
            if changed:
                bb.instructions = out


def _patched_drain_and_barrier(self, tick_clock, wait_clock):
    nc = self.nc
    drain_inst = nc.sync.drain()
    wait_clock.add_sem_waits(
        drain_inst.ins, ScopedClock({None: tick_clock.global_clock})
    )
    nc.all_engine_barrier()
    assert self.sems is not None
    popped = nc._tile_sem_poison_stack.pop()
    assert popped is self._sem_poison
    nc.clear_and_free_semaphores(list(self.sems.allocated().values()))
    nc.all_engine_barrier()
    _split_waits(nc)


tile.TileContext._drain_and_barrier = _patched_drain_and_barrier

_NC_CACHE = None
RUN_KWARGS: dict = {}
LAST_RES = None


def _build():
    nc = bass.Bass("TRN2", target_bir_lowering=False, debug=False)

    z1_d = nc.dram_tensor("z1", [N, D], BF16, kind="ExternalInput").ap()
    z2_d = nc.dram_tensor("z2", [N, D], BF16, kind="ExternalInput").ap()
    w1_d = nc.dram_tensor("w1", [D, D], BF16, kind="ExternalInput").ap()
    w2_d = nc.dram_tensor("w2", [D, D], BF16, kind="ExternalInput").ap()
    b1_d = nc.dram_tensor("b1", [D, 1], F32, kind="ExternalInput").ap()
    b2p_d = nc.dram_tensor("b2p", [D, 1], F32, kind="ExternalInput").ap()

    rs_d = [
        nc.dram_tensor(f"rs{i}", [128, 16], F32, kind="ExternalOutput").ap()
        for i in range(3)
    ]
    cs12_d = nc.dram_tensor("cs12", [1, N], F32, kind="ExternalOutput").ap()
    num_d = nc.dram_tensor("num", [1, R], F32, kind="ExternalOutput").ap()

    with tile.TileContext(nc) as tc:
        with (
            tc.tile_pool(name="persist", bufs=1) as pers,
            tc.tile_pool(name="consts", bufs=1) as consts,
        ):
            anT = pers.tile([128, N], BF16, tag="anT")
            bnT = pers.tile([128, N], BF16, tag="bnT")
            rs_sb = [
                pers.tile([128, 16], F32, tag=f"rs{i}", name=f"rs_sb{i}")
                for i in range(3)
            ]

            ident = consts.tile([128, 128], BF16, tag="ident")
            make_identity(nc, ident[:])
            ones_col_bf = consts.tile([128, 1], BF16, tag="ocb")
            nc.gpsimd.memset(ones_col_bf[:], 1.0)
            ones_col_f = consts.tile([128, 1], F32, tag="ocf")
            nc.gpsimd.memset(ones_col_f[:], 1.0)
            ones_row_f = consts.tile([1, 128], F32, tag="orf")
            nc.gpsimd.memset(ones_row_f[:], 1.0)
            w1sb = consts.tile([128, 128], BF16, tag="w1")
            nc.sync.dma_start(w1sb[:], w1_d[:])
            w2sb = consts.tile([128, 128], BF16, tag="w2")
            nc.sync.dma_start(w2sb[:], w2_d[:])
            b1sb = consts.tile([128, 1], F32, tag="b1")
            nc.sync.dma_start(b1sb[:], b1_d[:])
            b2psb = consts.tile([128, 1], F32, tag="b2p")
            nc.sync.dma_start(b2psb[:], b2p_d[:])

            # ---------------- setup: projections + normalize ----------------
            with (
                tc.tile_pool(name="szt", bufs=2) as szt,
                tc.tile_pool(name="sw", bufs=4) as sw,
                tc.tile_pool(name="sp2", bufs=2, space="PSUM") as sp2,
                tc.tile_pool(name="sp1", bufs=1, space="PSUM") as sp1,
            ):
                for t, (z_d, aT) in enumerate([(z1_d, anT), (z2_d, bnT)]):
                    zT = szt.tile([128, N], BF16, tag="zT")
                    # transpose z into [d, i] layout via PE
                    for i in range(N // 128):
                        nat = sw.tile([128, 128], BF16, tag="nat")
                        nc.sync.dma_start(nat[:], z_d[i * 128 : (i + 1) * 128, :])
                        tps = sp1.tile([128, 128], BF16, tag="tps")
                        nc.tensor.transpose(tps[:], nat[:], ident[:])
                        nc.vector.tensor_copy(zT[:, i * 128 : (i + 1) * 128], tps[:])
                    # project + normalize, 512-wide chunks
                    for k in range(N // 512):
                        sl = slice(k * 512, (k + 1) * 512)
                        psA = sp2.tile([128, 512], F32, tag="psA")
                        nc.tensor.matmul(psA[:], w1sb[:], zT[:, sl])
                        expu = sw.tile([128, 512], F32, tag="expu")
                        nc.scalar.activation(expu[:], psA[:], AF.Exp, bias=b1sb[:])
                        relu = sw.tile([128, 512], F32, tag="relu")
                        nc.scalar.activation(relu[:], psA[:], AF.Relu, bias=b1sb[:])
                        # elu(y)+1 = min(exp(y),1) + max(y,0)
                        p1c = sw.tile([128, 512], BF16, tag="p1c")
                        nc.vector.scalar_tensor_tensor(
                            p1c[:], expu[:], 1.0, relu[:], OP.min, OP.add
                        )
                        psB = sp2.tile([128, 512], F32, tag="psB")
                        nc.tensor.matmul(psB[:], w2sb[:], p1c[:])
                        hc = sw.tile([128, 512], BF16, tag="hc")
                        nc.vector.tensor_scalar(hc[:], psB[:], b2psb[:], None, OP.add)
                        sq = sw.tile([128, 512], BF16, tag="sq")
                        nc.vector.tensor_mul(sq[:], hc[:], hc[:])
                        psC = sp1.tile([1, 512], F32, tag="psC")
                        nc.tensor.matmul(psC[:], ones_col_bf[:], sq[:])
                        lnq = sw.tile([1, 512], F32, tag="lnq")
                        nc.scalar.activation(lnq[:], psC[:], AF.Ln)
                        psD = sp2.tile([128, 512], F32, tag="psD")
                        nc.tensor.matmul(psD[:], ones_row_f[:], lnq[:])
                        invnb = sw.tile([128, 512], F32, tag="invnb")
                        nc.scalar.activation(invnb[:], psD[:], AF.Exp, scale=-0.5)
                        nc.vector.tensor_mul(aT[:, sl], invnb[:], hc[:])

                # num_i = exp(an_i . bn_i / tau) for local rows (cols 0..R)
                for q in range(R // 512):
                    sl = slice(q * 512, (q + 1) * 512)
                    prod = sw.tile([128, 512], F32, tag="prod")
                    nc.vector.tensor_mul(prod[:], anT[:, sl], bnT[:, sl])
                    psN = sp1.tile([1, 512], F32, tag="psC")
                    nc.tensor.matmul(psN[:], ones_col_f[:], prod[:])
                    numt = sw.tile([1, 512], F32, tag="numt")
                    nc.scalar.activation(numt[:], psN[:], AF.Exp, scale=INV_TAU)
                    nc.sync.dma_start(num_d[0:1, sl], numt[:])

            # ---------------- main loop: 3 similarity row-blocks ------------
            with tc.tile_pool(name="mp", bufs=1) as mp:
                colacc = mp.tile([128, N], F32, tag="colacc")
                nc.gpsimd.memset(colacc[:], 0.0)
                with (
                    tc.tile_pool(name="me", bufs=4) as me,
                    tc.tile_pool(name="ma", bufs=4) as ma,
                    tc.tile_pool(name="mpp", bufs=2, space="PSUM") as mpp,
                ):
                    mats = [(anT, anT, False), (anT, bnT, True), (bnT, bnT, False)]
                    for mi, (lhs, rhs, need_col) in enumerate(mats):
                        for m in range(R // 128):
                            lT = lhs[:, m * 128 : (m + 1) * 128]
                            acc8 = ma.tile([128, 8], F32, tag="acc8")
                            for jt in range(8):
                                ps = mpp.tile([128, 2048], F32, tag="mm")
                                for q in range(4):
                                    nc.tensor.matmul(
                                        ps[:, q * 512 : (q + 1) * 512],
                                        lT,
                                        rhs[:, jt * 2048 + q * 512 : jt * 2048 + (q + 1) * 512],
                                    )
                                E = me.tile([128, 2048], BF16, tag="E")
                                nc.scalar.activation(
                                    E[:],
                                    ps[:],
                                    AF.Exp,
                                    scale=INV_TAU,
                                    accum_out=acc8[:, jt : jt + 1],
                                )
                                if need_col:
                                    csl = slice(jt * 2048, (jt + 1) * 2048)
                                    nc.vector.scalar_tensor_tensor(
                                        colacc[:, csl], E[:], 1.0, colacc[:, csl],
                                        OP.mult, OP.add,
                                    )
                            nc.vector.tensor_reduce(
                                rs_sb[mi][:, m : m + 1], acc8[:],
                                mybir.AxisListType.X, OP.add,
                            )
                        nc.sync.dma_start(rs_d[mi][:], rs_sb[mi][:])

                # cs12[j] = sum over this core's rows of exp(S12)[.,j]
                with (
                    tc.tile_pool(name="cw", bufs=2) as cw,
                    tc.tile_pool(name="cpp", bufs=2, space="PSUM") as cpp,
                ):
                    for k in range(N // 512):
                        sl = slice(k * 512, (k + 1) * 512)
                        psK = cpp.tile([1, 512], F32, tag="psK")
                        nc.tensor.matmul(psK[:], ones_col_f[:], colacc[:, sl])
                        cst = cw.tile([1, 512], F32, tag="cst")
                        nc.vector.tensor_copy(cst[:], psK[:])
                        nc.sync.dma_start(cs12_d[0:1, sl], cst[:])

    return nc


def _get_nc():
    global _NC_CACHE
    if _NC_CACHE is None:
        _NC_CACHE = _build()
    return _NC_CACHE


def kernel(z1, z2, W1, b1, W2, b2):
    global LAST_RES
    bf = ml_dtypes.bfloat16
    z1 = np.asarray(z1, dtype=np.float32)
    z2 = np.asarray(z2, dtype=np.float32)
    W1 = np.asarray(W1, dtype=np.float32)
    W2 = np.asarray(W2, dtype=np.float32)
    b1 = np.asarray(b1, dtype=np.float32)
    b2 = np.asarray(b2, dtype=np.float32)
    # fold the "-1" of elu(y) = (min(exp y,1)+max(y,0)) - 1 into the 2nd bias
    b2p = (b2.astype(np.float64) - W2.astype(np.float64).sum(0)).astype(np.float32)

    nc = _get_nc()
    in_maps = []
    for c in range(NCORES):
        in_maps.append(
            {
                "z1": np.roll(z1, -c * R, axis=0).astype(bf),
                "z2": np.roll(z2, -c * R, axis=0).astype(bf),
                "w1": W1.astype(bf),
                "w2": W2.astype(bf),
                "b1": b1.reshape(D, 1).copy(),
                "b2p": b2p.reshape(D, 1).copy(),
            }
        )
    res = run_bass_kernel_spmd(nc, in_maps, list(range(NCORES)), **RUN_KWARGS)
    LAST_RES = res

    e2 = np.exp(np.float64(INV_TAU))
    rs11 = np.empty(N, np.float64)
    rs12 = np.empty(N, np.float64)
    rs22 = np.empty(N, np.float64)
    num = np.empty(N, np.float64)
    cs12 = np.zeros(N, np.float64)
    for c in range(NCORES):
        r = res.results[c]
        sl = slice(c * R, (c + 1) * R)
        rs11[sl] = r["rs0"].astype(np.float64).T.reshape(R)
        rs12[sl] = r["rs1"].astype(np.float64).T.reshape(R)
        rs22[sl] = r["rs2"].astype(np.float64).T.reshape(R)
        num[sl] = r["num"].astype(np.float64).reshape(R)
        cs12 += np.roll(r["cs12"].astype(np.float64).reshape(N), c * R)

    den1 = rs11 + rs12 - e2
    den2 = rs22 + cs12 - e2
    l1 = np.log(den1) - np.log(num)
    l2 = np.log(den2) - np.log(num)
    loss = np.mean(0.5 * (l1 + l2))
    return np.array(loss, dtype=np.float32)



# revision 4
# speedup vs baseline: 1.9745x; 1.9745x over previous
"""Trainium2 Bass kernel for the HGCA contrastive loss (nn_HGCA_10857677324785).

loss = mean_i 0.5*(l1_i + l2_i),
  l1_i = log(den1_i) - log(num_i), den1_i = sum_j e^{2 an_i.an_j} + sum_j e^{2 an_i.bn_j} - e^2
  l2_i = log(den2_i) - log(num_i), den2_i = sum_j e^{2 bn_i.bn_j} + sum_j e^{2 bn_i.an_j} - e^2
where an/bn are the L2-normalized projections elu-MLP(z1/z2).

Distribution: the projections (O(N D^2), 0.5% of the FLOPs) are computed on
the host in f32 — the sharding hint's "all-gathered normalized projections"
— and handed to every core pre-transposed ([d, i] layout, bf16) and
row-rolled so each core's 2048 rows sit at local columns [0, 2048).

Each core computes its row-block of the three N x N similarity exps.  The
symmetric matrices S11 = an@an.T and S22 = bn@bn.T are only half-computed:
each 128-row tile m processes a diagonal-anchored window of 64 column tiles
(local cols [m*128, m*128+8192)) plus the distance-64 tile as a separate
"band" pass.  Row sums come from the ACT engine's fused accumulator; column
sums of the D in [1,63] part are accumulated in bf16 on the DVE and exported
raw — by symmetry they are exactly the row-sum contributions of the
uncomputed distance >= 65 tiles.  S12 is not symmetric: full rows with both
row sums (ACT accum) and bf16 column accumulation (DVE).  The host sums the
raw column accumulators over partitions, rolls them into global row space,
and assembles the scalar loss in f64 (log num_i = 2 an_i.bn_i directly).
"""

import ml_dtypes
import numpy as np

import concourse.bass as bass
import concourse.tile as tile
from concourse import mybir
from concourse.bass_utils import run_bass_kernel_spmd

N = 16384
D = 128
NCORES = 8
R = N // NCORES  # 2048 rows per core
TILES = R // 128  # 16 row tiles per core
WIN = 8192  # window: distance tiles 0..63
CHUNK = 2048  # psum/exp chunk width
CA_COLS = 15 * 128 + WIN - 128  # 9984: colacc for D in [1,63]
ANT_COLS = 15 * 128 + WIN + 2048  # 10240: rightmost anT column ever read
INV_TAU = 2.0  # 1/0.5
F32 = mybir.dt.float32
BF16 = mybir.dt.bfloat16
AF = mybir.ActivationFunctionType
OP = mybir.AluOpType

# This walrus build supports at most 2 sync waits per instruction; Tile's sem
# assignment freely emits 3-11. Post-pass: hoist excess waits onto injected
# same-engine EventSemaphore fillers (engine queues are FIFO, so waits on an
# earlier filler happen-before the original instruction executes).

_MAX_WAITS = 1


def _split_waits(nc):
    for fn in nc.m.functions:
        for bb in fn.blocks:
            insts = list(bb.instructions)
            out = []
            changed = False
            for inst in insts:
                si = inst.sync_info
                w = list(si.on_wait) if si and si.on_wait else []
                if len(w) > _MAX_WAITS:
                    changed = True
                    extra, keep = w[:-_MAX_WAITS], w[-_MAX_WAITS:]
                    for i in range(0, len(extra), _MAX_WAITS):
                        f = mybir.InstEventSemaphore(
                            name=f"{inst.name}_wsplit{i}",
                            engine=inst.engine,
                            ins=[],
                            outs=[],
                            sync_info=mybir.SyncInfo(
                                on_wait=extra[i : i + _MAX_WAITS], on_update=[]
                            ),
                        )
                        out.append(f)
                    inst.sync_info = mybir.SyncInfo(
                        on_wait=keep,
                        on_update=list(si.on_update) if si.on_update else [],
                    )
                out.append(inst)
            if changed:
                bb.instructions = out


def _patched_drain_and_barrier(self, tick_clock, wait_clock):
    from concourse.vector_clock import ScopedClock

    nc = self.nc
    drain_inst = nc.sync.drain()
    wait_clock.add_sem_waits(
        drain_inst.ins, ScopedClock({None: tick_clock.global_clock})
    )
    nc.all_engine_barrier()
    assert self.sems is not None
    popped = nc._tile_sem_poison_stack.pop()
    assert popped is self._sem_poison
    nc.clear_and_free_semaphores(list(self.sems.allocated().values()))
    nc.all_engine_barrier()
    _split_waits(nc)


tile.TileContext._drain_and_barrier = _patched_drain_and_barrier

_NC_CACHE = None
RUN_KWARGS: dict = {}
LAST_RES = None


def _build():
    nc = bass.Bass("TRN2", target_bir_lowering=False, debug=False)

    anT_d = nc.dram_tensor("anT", [128, ANT_COLS], BF16, kind="ExternalInput").ap()
    bnT_d = nc.dram_tensor("bnT", [128, N], BF16, kind="ExternalInput").ap()

    acc11_d = nc.dram_tensor("acc11", [128, 4 * TILES], F32, kind="ExternalOutput").ap()
    acc22_d = nc.dram_tensor("acc22", [128, 4 * TILES], F32, kind="ExternalOutput").ap()
    acc12_d = nc.dram_tensor("acc12", [128, 8 * TILES], F32, kind="ExternalOutput").ap()
    band11_d = nc.dram_tensor("band11", [128, R], BF16, kind="ExternalOutput").ap()
    band22_d = nc.dram_tensor("band22", [128, R], BF16, kind="ExternalOutput").ap()
    ca11_d = nc.dram_tensor("ca11", [128, CA_COLS], BF16, kind="ExternalOutput").ap()
    ca22_d = nc.dram_tensor("ca22", [128, CA_COLS], BF16, kind="ExternalOutput").ap()
    ca12_d = nc.dram_tensor("ca12", [128, N], BF16, kind="ExternalOutput").ap()

    with tile.TileContext(nc) as tc:
        with tc.tile_pool(name="pers", bufs=1) as pers:
            anT = pers.tile([128, ANT_COLS], BF16, tag="anT")
            bnT = pers.tile([128, N], BF16, tag="bnT")
            ca11 = pers.tile([128, CA_COLS], BF16, tag="ca11")
            ca22 = pers.tile([128, CA_COLS], BF16, tag="ca22")
            ca12 = pers.tile([128, N], BF16, tag="ca12")
            acc11 = pers.tile([128, 4 * TILES], F32, tag="acc11")
            acc22 = pers.tile([128, 4 * TILES], F32, tag="acc22")
            acc12 = pers.tile([128, 8 * TILES], F32, tag="acc12")

            # input DMAs, chunked so the first window can start early
            for c0 in range(0, ANT_COLS, 4096):
                c1 = min(c0 + 4096, ANT_COLS)
                nc.sync.dma_start(anT[:, c0:c1], anT_d[:, c0:c1])
            for c0 in range(0, N, 4096):
                nc.sync.dma_start(bnT[:, c0 : c0 + 4096], bnT_d[:, c0 : c0 + 4096])

            with (
                tc.tile_pool(name="mp", bufs=2, space="PSUM") as mp,
                tc.tile_pool(name="ep", bufs=4) as ep,
            ):
                mats = [
                    (anT, anT, acc11, acc11_d, ca11, ca11_d, band11_d, True),
                    (bnT, bnT, acc22, acc22_d, ca22, ca22_d, band22_d, True),
                    (anT, bnT, acc12, acc12_d, ca12, ca12_d, None, False),
                ]
                for lhs, rhs, acc, acc_d, ca, ca_d, band_d, sym in mats:
                    nch = 4 if sym else 8
                    for m in range(TILES):
                        lT = lhs[:, m * 128 : (m + 1) * 128]
                        base = m * 128 if sym else 0
                        for k in range(nch):
                            c0 = base + k * CHUNK
                            ps = mp.tile([128, CHUNK], F32, tag="mm")
                            for q in range(4):
                                nc.tensor.matmul(
                                    ps[:, q * 512 : (q + 1) * 512],
                                    lT,
                                    rhs[:, c0 + q * 512 : c0 + (q + 1) * 512],
                                )
                            E = ep.tile([128, CHUNK], BF16, tag="E")
                            slot = m * nch + k
                            nc.scalar.activation(
                                E[:],
                                ps[:],
                                AF.Exp,
                                scale=INV_TAU,
                                accum_out=acc[:, slot : slot + 1],
                            )
                            # column accumulation (bf16, DVE 2x mode).
                            # sym: region D in [1,63] = cols [m*128+128, m*128+8192)
                            #   -> ca index range [m*128, m*128+8064)
                            # full (S12): all cols, ca index = col
                            if sym:
                                # chunk cols [c0, c0+2048); region starts at m*128+128
                                lo = max(c0, m * 128 + 128)
                                hi = c0 + CHUNK
                                e0 = lo - c0
                                # ca idx of col x = x - 128
                                a0 = lo - 128
                                a1 = hi - 128
                                # columns >= prev tile's end are first-touched
                                new0 = 128 if m == 0 else m * 128 + 8064
                                if lo >= new0:
                                    nc.vector.tensor_copy(ca[:, a0:a1], E[:, e0:CHUNK])
                                elif hi <= new0:
                                    nc.vector.tensor_tensor(
                                        ca[:, a0:a1], E[:, e0:CHUNK], ca[:, a0:a1], OP.add
                                    )
                                else:
                                    sp = new0 - c0
                                    nc.vector.tensor_tensor(
                                        ca[:, a0 : new0 - 128],
                                        E[:, e0:sp],
                                        ca[:, a0 : new0 - 128],
                                        OP.add,
                                    )
                                    nc.vector.tensor_copy(
                                        ca[:, new0 - 128 : a1], E[:, sp:CHUNK]
                                    )
                            else:
                                if m == 0:
                                    nc.vector.tensor_copy(ca[:, c0 : c0 + CHUNK], E[:])
                                else:
                                    nc.vector.tensor_tensor(
                                        ca[:, c0 : c0 + CHUNK],
                                        E[:],
                                        ca[:, c0 : c0 + CHUNK],
                                        OP.add,
                                    )
                    if sym:
                        # band pass: distance-64 tiles (m, m+64), rowsum-only,
                        # raw exps exported; host reduces.
                        ps = mp.tile([128, CHUNK], F32, tag="mm")
                        for m in range(TILES):
                            nc.tensor.matmul(
                                ps[:, m * 128 : (m + 1) * 128],
                                lhs[:, m * 128 : (m + 1) * 128],
                                rhs[:, WIN + m * 128 : WIN + (m + 1) * 128],
                            )
                        Eb = ep.tile([128, CHUNK], BF16, tag="E")
                        nc.scalar.activation(Eb[:], ps[:], AF.Exp, scale=INV_TAU)
                        nc.sync.dma_start(band_d[:, :], Eb[:])
                    nc.sync.dma_start(acc_d[:, :], acc[:, :])
                    cols = CA_COLS if sym else N
                    for c0 in range(0, cols, 4096):
                        c1 = min(c0 + 4096, cols)
                        nc.sync.dma_start(ca_d[:, c0:c1], ca[:, c0:c1])

    return nc


def _get_nc():
    global _NC_CACHE
    if _NC_CACHE is None:
        _NC_CACHE = _build()
    return _NC_CACHE


def _project(z, W1, b1, W2, b2):
    u = z @ W1 + b1
    h = np.where(u > 0, u, np.expm1(np.minimum(u, 0.0))) @ W2 + b2
    n = np.sqrt(np.sum(h * h, axis=1, keepdims=True))
    return h / np.maximum(n, 1e-12)


def kernel(z1, z2, W1, b1, W2, b2):
    global LAST_RES
    bf = ml_dtypes.bfloat16
    z1 = np.asarray(z1, dtype=np.float32)
    z2 = np.asarray(z2, dtype=np.float32)
    W1 = np.asarray(W1, dtype=np.float32)
    W2 = np.asarray(W2, dtype=np.float32)
    b1 = np.asarray(b1, dtype=np.float32)
    b2 = np.asarray(b2, dtype=np.float32)

    an = _project(z1, W1, b1, W2, b2)
    bn = _project(z2, W1, b1, W2, b2)
    anT_bf = np.ascontiguousarray(an.T).astype(bf)  # [128, N]
    bnT_bf = np.ascontiguousarray(bn.T).astype(bf)

    nc = _get_nc()
    in_maps = []
    for c in range(NCORES):
        a = np.roll(anT_bf, -c * R, axis=1)
        b = np.roll(bnT_bf, -c * R, axis=1)
        in_maps.append(
            {
                "anT": np.ascontiguousarray(a[:, :ANT_COLS]),
                "bnT": np.ascontiguousarray(b),
            }
        )
    res = run_bass_kernel_spmd(nc, in_maps, list(range(NCORES)), **RUN_KWARGS)
    LAST_RES = res

    e2 = np.exp(np.float64(INV_TAU))
    den1 = np.zeros(N, np.float64)
    den2 = np.zeros(N, np.float64)
    idx_ca = None
    for c in range(NCORES):
        r = res.results[c]
        own = slice(c * R, (c + 1) * R)
        # windowed row sums: acc[p, m*nch+k] for row m*128+p
        a11 = r["acc11"].astype(np.float64).reshape(128, TILES, 4)
        a22 = r["acc22"].astype(np.float64).reshape(128, TILES, 4)
        a12 = r["acc12"].astype(np.float64).reshape(128, TILES, 8)
        rs11 = a11.sum(axis=2).T.reshape(R)  # [m,p] -> row m*128+p
        rs22 = a22.sum(axis=2).T.reshape(R)
        rs12 = a12.sum(axis=2).T.reshape(R)
        # band: E[p, m*128+j] = exp tile (m, m+64) -> row m*128+p sums over j
        b11 = r["band11"].astype(np.float64).reshape(128, TILES, 128)
        b22 = r["band22"].astype(np.float64).reshape(128, TILES, 128)
        rs11 += b11.sum(axis=2).T.reshape(R)
        rs22 += b22.sum(axis=2).T.reshape(R)
        den1[own] += rs11 + rs12
        den2[own] += rs22
        # column accumulators: partition-sum then roll to global rows
        cs11 = r["ca11"].astype(np.float64).sum(axis=0)  # local col j+128
        cs22 = r["ca22"].astype(np.float64).sum(axis=0)
        cs12 = r["ca12"].astype(np.float64).sum(axis=0)  # local col j
        if idx_ca is None:
            idx_ca = np.arange(CA_COLS)
        den1[(c * R + 128 + idx_ca) % N] += cs11
        den2[(c * R + 128 + idx_ca) % N] += cs22
        den2[(c * R + np.arange(N)) % N] += cs12
    den1 -= e2
    den2 -= e2

    lognum = 2.0 * np.sum(an.astype(np.float64) * bn.astype(np.float64), axis=1)
    loss = np.mean(0.5 * (np.log(den1) + np.log(den2)) - lognum)
    return np.array(loss, dtype=np.float32)


# revision 13
# speedup vs baseline: 2.2169x; 1.1228x over previous
"""Trainium2 Bass kernel for the HGCA contrastive loss (nn_HGCA_10857677324785).

loss = mean_i 0.5*(l1_i + l2_i),
  l1_i = log(den1_i) - log(num_i), den1_i = sum_j e^{2 an_i.an_j} + sum_j e^{2 an_i.bn_j} - e^2
  l2_i = log(den2_i) - log(num_i), den2_i = sum_j e^{2 bn_i.bn_j} + sum_j e^{2 bn_i.an_j} - e^2
where an/bn are the L2-normalized projections elu-MLP(z1/z2).

Distribution: the projections (O(N D^2), 0.5% of the FLOPs) are computed on
the host in f32 — the sharding hint's "all-gathered normalized projections"
— and handed to every core pre-transposed ([d, i] layout, bf16) and
row-rolled so each core's 2048 rows sit at local columns [0, 2048).

Each core computes its row-block of the three N x N similarity exps.  The
symmetric matrices S11 = an@an.T and S22 = bn@bn.T are only half-computed:
each 128-row tile m processes a diagonal-anchored window of 64 column tiles
(local cols [m*128, m*128+8192)) plus the distance-64 tile as a separate
"band" pass.  Row sums come from the ACT engine's fused accumulator; column
sums of the D in [1,63] part are accumulated in bf16 on the DVE and exported
raw — by symmetry they are exactly the row-sum contributions of the
uncomputed distance >= 65 tiles.  S12 is not symmetric: full rows with both
row sums (ACT accum) and bf16 column accumulation (DVE).  The host sums the
raw column accumulators over partitions, rolls them into global row space,
and assembles the scalar loss in f64 (log num_i = 2 an_i.bn_i directly).
"""

import ml_dtypes
import numpy as np

import concourse.bass as bass
import concourse.tile as tile
from concourse import mybir
from concourse.bass_utils import run_bass_kernel_spmd

N = 16384
D = 128
NCORES = 8
R = N // NCORES  # 2048 rows per core
TILES = R // 128  # 16 row tiles per core
WIN = 8192  # window: distance tiles 0..63
CHUNK = 2048  # psum/exp chunk width
CA_COLS = 15 * 128 + WIN - 128  # 9984: colacc for D in [1,63]
ANT_COLS = 15 * 128 + WIN + 2048  # 10240: rightmost anT column ever read
INV_TAU = 2.0  # 1/0.5
F32 = mybir.dt.float32
BF16 = mybir.dt.bfloat16
I16 = mybir.dt.int16
AF = mybir.ActivationFunctionType
OP = mybir.AluOpType

# Schraudolph fast-exp on the DVE: I = int16(A*s + B); bf16-bits(I) ~ exp(2s).
# A folds in 1/tau; B calibrated for zero mean multiplicative bias over the
# (near-uniform) mantissa phase.  Offloads ACT-engine exp work per chunk.
SCHRA_A = 2.0 * 128.0 / float(np.log(2.0))
SCHRA_B = 16250.0
OFF_NUM, OFF_DEN = 75, 256  # fraction of main chunks exp'd on DVE (Bresenham)
DVE12_K = 3  # S12 chunks k < this accumulate columns on DVE (SBUF); rest SWDGE

# This walrus build supports at most 2 sync waits per instruction; Tile's sem
# assignment freely emits 3-11. Post-pass: hoist excess waits onto injected
# same-engine EventSemaphore fillers (engine queues are FIFO, so waits on an
# earlier filler happen-before the original instruction executes).

_MAX_WAITS = 1


def _split_waits(nc):
    for fn in nc.m.functions:
        for bb in fn.blocks:
            insts = list(bb.instructions)
            out = []
            changed = False
            for inst in insts:
                si = inst.sync_info
                w = list(si.on_wait) if si and si.on_wait else []
                if len(w) > _MAX_WAITS:
                    changed = True
                    extra, keep = w[:-_MAX_WAITS], w[-_MAX_WAITS:]
                    for i in range(0, len(extra), _MAX_WAITS):
                        f = mybir.InstEventSemaphore(
                            name=f"{inst.name}_wsplit{i}",
                            engine=inst.engine,
                            ins=[],
                            outs=[],
                            sync_info=mybir.SyncInfo(
                                on_wait=extra[i : i + _MAX_WAITS], on_update=[]
                            ),
                        )
                        out.append(f)
                    inst.sync_info = mybir.SyncInfo(
                        on_wait=keep,
                        on_update=list(si.on_update) if si.on_update else [],
                    )
                out.append(inst)
            if changed:
                bb.instructions = out


def _patched_drain_and_barrier(self, tick_clock, wait_clock):
    from concourse.vector_clock import ScopedClock

    nc = self.nc
    drain_inst = nc.sync.drain()
    wait_clock.add_sem_waits(
        drain_inst.ins, ScopedClock({None: tick_clock.global_clock})
    )
    nc.all_engine_barrier()
    assert self.sems is not None
    popped = nc._tile_sem_poison_stack.pop()
    assert popped is self._sem_poison
    nc.clear_and_free_semaphores(list(self.sems.allocated().values()))
    nc.all_engine_barrier()
    _split_waits(nc)


tile.TileContext._drain_and_barrier = _patched_drain_and_barrier

_NC_CACHE = None
RUN_KWARGS: dict = {}
LAST_RES = None


def _build():
    nc = bass.Bass("TRN2", target_bir_lowering=False, debug=False)

    anT_d = nc.dram_tensor("anT", [128, ANT_COLS], BF16, kind="ExternalInput").ap()
    bnT_d = nc.dram_tensor("bnT", [128, N], BF16, kind="ExternalInput").ap()

    acc11_d = nc.dram_tensor("acc11", [128, 4 * TILES], F32, kind="ExternalOutput").ap()
    acc22_d = nc.dram_tensor("acc22", [128, 4 * TILES], F32, kind="ExternalOutput").ap()
    acc12_d = nc.dram_tensor("acc12", [128, 8 * TILES], F32, kind="ExternalOutput").ap()
    band11_d = nc.dram_tensor("band11", [128, R], BF16, kind="ExternalOutput").ap()
    band22_d = nc.dram_tensor("band22", [128, R], BF16, kind="ExternalOutput").ap()
    ca11_d = nc.dram_tensor("ca11", [128, CA_COLS], BF16, kind="ExternalOutput").ap()
    ca22_d = nc.dram_tensor("ca22", [128, CA_COLS], BF16, kind="ExternalOutput").ap()
    ca12_d = nc.dram_tensor("ca12", [128, N], BF16, kind="ExternalOutput").ap()

    with tile.TileContext(nc) as tc:
        with tc.tile_pool(name="pers", bufs=1) as pers:
            anT = pers.tile([128, ANT_COLS], BF16, tag="anT")
            bnT = pers.tile([128, N], BF16, tag="bnT")
            ca12sb = pers.tile([128, DVE12_K * CHUNK], BF16, tag="ca12sb")
            acc11 = pers.tile([128, 4 * TILES], F32, tag="acc11")
            acc22 = pers.tile([128, 4 * TILES], F32, tag="acc22")
            acc12 = pers.tile([128, 8 * TILES], F32, tag="acc12")

            # input DMAs, chunked so the first window can start early
            for c0 in range(0, ANT_COLS, 4096):
                c1 = min(c0 + 4096, ANT_COLS)
                nc.sync.dma_start(anT[:, c0:c1], anT_d[:, c0:c1])
            for c0 in range(0, N, 4096):
                nc.sync.dma_start(bnT[:, c0 : c0 + 4096], bnT_d[:, c0 : c0 + 4096])

            with (
                tc.tile_pool(name="mp", bufs=2, space="PSUM") as mp,
                tc.tile_pool(name="ep", bufs=6) as ep,
            ):
                mats = [
                    (anT, anT, acc11, acc11_d, ca11_d, band11_d, True),
                    (bnT, bnT, acc22, acc22_d, ca22_d, band22_d, True),
                    (anT, bnT, acc12, acc12_d, ca12_d, None, False),
                ]
                noff = [0, 0]  # Bresenham state: [chunks seen, chunks offloaded]
                for lhs, rhs, acc, acc_d, ca_d, band_d, sym in mats:
                    nch = 4 if sym else 8
                    for m in range(TILES):
                        lT = lhs[:, m * 128 : (m + 1) * 128]
                        base = m * 128 if sym else 0
                        for k in range(nch):
                            c0 = base + k * CHUNK
                            ps = mp.tile([128, CHUNK], F32, tag="mm")
                            for q in range(4):
                                nc.tensor.matmul(
                                    ps[:, q * 512 : (q + 1) * 512],
                                    lT,
                                    rhs[:, c0 + q * 512 : c0 + (q + 1) * 512],
                                )
                            E = ep.tile([128, CHUNK], BF16, tag="E")
                            slot = m * nch + k
                            noff[0] += 1
                            off = noff[0] * OFF_NUM // OFF_DEN > noff[1]
                            if off:
                                noff[1] += 1
                                nc.vector.tensor_scalar(
                                    E[:].bitcast(I16),
                                    ps[:],
                                    SCHRA_A,
                                    SCHRA_B,
                                    OP.mult,
                                    OP.add,
                                )
                                nc.vector.tensor_reduce(
                                    acc[:, slot : slot + 1],
                                    E[:],
                                    mybir.AxisListType.X,
                                    OP.add,
                                )
                            else:
                                nc.scalar.activation(
                                    E[:],
                                    ps[:],
                                    AF.Exp,
                                    scale=INV_TAU,
                                    accum_out=acc[:, slot : slot + 1],
                                )
                            # column accumulation:
                            # sym: region D in [1,63] = cols [m*128+128, m*128+8192)
                            #   -> ca idx [m*128, m*128+8064), SWDGE accum to DRAM
                            # S12: k < DVE12_K on DVE into SBUF, rest SWDGE to DRAM
                            if sym:
                                lo = max(c0, m * 128 + 128)
                                hi = c0 + CHUNK
                                e0 = lo - c0
                                a0 = lo - 128
                                a1 = hi - 128
                                # columns >= prev tile's end are first-touched
                                new0 = 128 if m == 0 else m * 128 + 8064
                                if lo >= new0:
                                    nc.gpsimd.dma_start(ca_d[:, a0:a1], E[:, e0:CHUNK])
                                elif hi <= new0:
                                    nc.gpsimd.dma_start(
                                        ca_d[:, a0:a1], E[:, e0:CHUNK], accum_op=OP.add
                                    )
                                else:
                                    sp = new0 - c0
                                    nc.gpsimd.dma_start(
                                        ca_d[:, a0 : new0 - 128],
                                        E[:, e0:sp],
                                        accum_op=OP.add,
                                    )
                                    nc.gpsimd.dma_start(
                                        ca_d[:, new0 - 128 : a1], E[:, sp:CHUNK]
                                    )
                            elif k < DVE12_K:
                                if m == 0:
                                    nc.vector.tensor_copy(ca12sb[:, c0 : c0 + CHUNK], E[:])
                                else:
                                    nc.vector.tensor_tensor(
                                        ca12sb[:, c0 : c0 + CHUNK],
                                        E[:],
                                        ca12sb[:, c0 : c0 + CHUNK],
                                        OP.add,
                                    )
                            else:
                                if m == 0:
                                    nc.gpsimd.dma_start(ca_d[:, c0 : c0 + CHUNK], E[:])
                                else:
                                    nc.gpsimd.dma_start(
                                        ca_d[:, c0 : c0 + CHUNK], E[:], accum_op=OP.add
                                    )
                    if sym:
                        # band pass: distance-64 tiles (m, m+64), rowsum-only,
                        # raw exps exported; host reduces.
                        ps = mp.tile([128, CHUNK], F32, tag="mm")
                        for m in range(TILES):
                            nc.tensor.matmul(
                                ps[:, m * 128 : (m + 1) * 128],
                                lhs[:, m * 128 : (m + 1) * 128],
                                rhs[:, WIN + m * 128 : WIN + (m + 1) * 128],
                            )
                        Eb = ep.tile([128, CHUNK], BF16, tag="E")
                        nc.scalar.activation(Eb[:], ps[:], AF.Exp, scale=INV_TAU)
                        nc.sync.dma_start(band_d[:, :], Eb[:])
                    nc.sync.dma_start(acc_d[:, :], acc[:, :])
                    if not sym:
                        for c0 in range(0, DVE12_K * CHUNK, CHUNK):
                            nc.sync.dma_start(
                                ca_d[:, c0 : c0 + CHUNK], ca12sb[:, c0 : c0 + CHUNK]
                            )

    return nc


def _get_nc():
    global _NC_CACHE
    if _NC_CACHE is None:
        _NC_CACHE = _build()
    return _NC_CACHE


def _project(z, W1, b1, W2, b2):
    u = z @ W1 + b1
    h = np.where(u > 0, u, np.expm1(np.minimum(u, 0.0))) @ W2 + b2
    n = np.sqrt(np.sum(h * h, axis=1, keepdims=True))
    return h / np.maximum(n, 1e-12)


def kernel(z1, z2, W1, b1, W2, b2):
    global LAST_RES
    bf = ml_dtypes.bfloat16
    z1 = np.asarray(z1, dtype=np.float32)
    z2 = np.asarray(z2, dtype=np.float32)
    W1 = np.asarray(W1, dtype=np.float32)
    W2 = np.asarray(W2, dtype=np.float32)
    b1 = np.asarray(b1, dtype=np.float32)
    b2 = np.asarray(b2, dtype=np.float32)

    an = _project(z1, W1, b1, W2, b2)
    bn = _project(z2, W1, b1, W2, b2)
    anT_bf = np.ascontiguousarray(an.T).astype(bf)  # [128, N]
    bnT_bf = np.ascontiguousarray(bn.T).astype(bf)

    nc = _get_nc()
    in_maps = []
    for c in range(NCORES):
        a = np.roll(anT_bf, -c * R, axis=1)
        b = np.roll(bnT_bf, -c * R, axis=1)
        in_maps.append(
            {
                "anT": np.ascontiguousarray(a[:, :ANT_COLS]),
                "bnT": np.ascontiguousarray(b),
            }
        )
    res = run_bass_kernel_spmd(nc, in_maps, list(range(NCORES)), **RUN_KWARGS)
    LAST_RES = res

    e2 = np.exp(np.float64(INV_TAU))
    den1 = np.zeros(N, np.float64)
    den2 = np.zeros(N, np.float64)
    idx_ca = None
    for c in range(NCORES):
        r = res.results[c]
        own = slice(c * R, (c + 1) * R)
        # windowed row sums: acc[p, m*nch+k] for row m*128+p
        a11 = r["acc11"].astype(np.float64).reshape(128, TILES, 4)
        a22 = r["acc22"].astype(np.float64).reshape(128, TILES, 4)
        a12 = r["acc12"].astype(np.float64).reshape(128, TILES, 8)
        rs11 = a11.sum(axis=2).T.reshape(R)  # [m,p] -> row m*128+p
        rs22 = a22.sum(axis=2).T.reshape(R)
        rs12 = a12.sum(axis=2).T.reshape(R)
        # band: E[p, m*128+j] = exp tile (m, m+64) -> row m*128+p sums over j
        b11 = r["band11"].astype(np.float64).reshape(128, TILES, 128)
        b22 = r["band22"].astype(np.float64).reshape(128, TILES, 128)
        rs11 += b11.sum(axis=2).T.reshape(R)
        rs22 += b22.sum(axis=2).T.reshape(R)
        den1[own] += rs11 + rs12
        den2[own] += rs22
        # column accumulators: partition-sum then roll to global rows
        cs11 = r["ca11"].astype(np.float64).sum(axis=0)  # local col j+128
        cs22 = r["ca22"].astype(np.float64).sum(axis=0)
        cs12 = r["ca12"].astype(np.float64).sum(axis=0)  # local col j
        if idx_ca is None:
            idx_ca = np.arange(CA_COLS)
        den1[(c * R + 128 + idx_ca) % N] += cs11
        den2[(c * R + 128 + idx_ca) % N] += cs22
        den2[(c * R + np.arange(N)) % N] += cs12
    den1 -= e2
    den2 -= e2

    lognum = 2.0 * np.sum(an.astype(np.float64) * bn.astype(np.float64), axis=1)
    loss = np.mean(0.5 * (np.log(den1) + np.log(den2)) - lognum)
    return np.array(loss, dtype=np.float32)


# revision 26
# speedup vs baseline: 2.5379x; 1.1448x over previous
"""Trainium2 Bass kernel for the HGCA contrastive loss (nn_HGCA_10857677324785).

loss = mean_i 0.5*(l1_i + l2_i),
  l1_i = log(den1_i) - log(num_i), den1_i = sum_j e^{2 an_i.an_j} + sum_j e^{2 an_i.bn_j} - e^2
  l2_i = log(den2_i) - log(num_i), den2_i = sum_j e^{2 bn_i.bn_j} + sum_j e^{2 bn_i.an_j} - e^2
where an/bn are the L2-normalized projections elu-MLP(z1/z2).

Distribution: the projections (O(N D^2), 0.5% of the FLOPs) are computed on
the host in f32 — the sharding hint's "all-gathered normalized projections"
— and handed to every core pre-transposed ([d, i] layout, bf16) and
row-rolled so each core's 2048 rows sit at local columns [0, 2048).

Each core computes its row-block of the three N x N similarity exps.  The
symmetric matrices S11 = an@an.T and S22 = bn@bn.T are only half-computed:
each 128-row tile m processes a diagonal-anchored window of 64 column tiles
(local cols [m*128, m*128+8192)) plus the distance-64 tile as a separate
"band" pass.  Row sums come from the ACT engine's fused accumulator; column
sums of the D in [1,63] part are accumulated in bf16 on the DVE and exported
raw — by symmetry they are exactly the row-sum contributions of the
uncomputed distance >= 65 tiles.  S12 is not symmetric: full rows with both
row sums (ACT accum) and bf16 column accumulation (DVE).  The host sums the
raw column accumulators over partitions, rolls them into global row space,
and assembles the scalar loss in f64 (log num_i = 2 an_i.bn_i directly).
"""

import ml_dtypes
import numpy as np

import concourse.bass as bass
import concourse.tile as tile
from concourse import mybir
from concourse.bass_utils import run_bass_kernel_spmd

N = 16384
D = 128
NCORES = 8
R = N // NCORES  # 2048 rows per core
TILES = R // 128  # 16 row tiles per core
WIN = 8192  # window: distance tiles 0..63
CHUNK = 1024  # psum/exp sub-chunk width (2-bank PSUM tiles, 4-deep rotation)
PAIR = 2048  # column-accumulate granularity (hw limit for accum DMAs)
CA_COLS = 15 * 128 + WIN - 128  # 9984: colacc for D in [1,63]
ANT_COLS = 15 * 128 + WIN + 2048  # 10240: rightmost anT column ever read
INV_TAU = 2.0  # 1/0.5
F32 = mybir.dt.float32
BF16 = mybir.dt.bfloat16
I16 = mybir.dt.int16
AF = mybir.ActivationFunctionType
OP = mybir.AluOpType

# Schraudolph fast-exp on the DVE: I = int16(A*s + B); bf16-bits(I) ~ exp(2s).
# A folds in 1/tau; B calibrated for zero mean multiplicative bias over the
# (near-uniform) mantissa phase.  Offloads ACT-engine exp work per chunk.
SCHRA_A = 2.0 * 128.0 / float(np.log(2.0))
SCHRA_B = 16250.0
OFF_NUM, OFF_DEN = 227, 512  # fraction of sub-chunks exp'd on DVE (Bresenham)
DVE12_P = 1  # S12 column pairs p < this accumulate on DVE (SBUF)
POOL12_P = 3  # S12 pairs p in [DVE12_P, this) accumulate on gpsimd (SBUF)
# S12 pairs p >= POOL12_P go per-pair SWDGE-accumulate to DRAM

# This walrus build supports at most 2 sync waits per instruction; Tile's sem
# assignment freely emits 3-11. Post-pass: hoist excess waits onto injected
# same-engine EventSemaphore fillers (engine queues are FIFO, so waits on an
# earlier filler happen-before the original instruction executes).

_MAX_WAITS = 1


def _split_waits(nc):
    for fn in nc.m.functions:
        for bb in fn.blocks:
            insts = list(bb.instructions)
            out = []
            changed = False
            for inst in insts:
                si = inst.sync_info
                w = list(si.on_wait) if si and si.on_wait else []
                if len(w) > _MAX_WAITS:
                    changed = True
                    extra, keep = w[:-_MAX_WAITS], w[-_MAX_WAITS:]
                    for i in range(0, len(extra), _MAX_WAITS):
                        f = mybir.InstEventSemaphore(
                            name=f"{inst.name}_wsplit{i}",
                            engine=inst.engine,
                            ins=[],
                            outs=[],
                            sync_info=mybir.SyncInfo(
                                on_wait=extra[i : i + _MAX_WAITS], on_update=[]
                            ),
                        )
                        out.append(f)
                    inst.sync_info = mybir.SyncInfo(
                        on_wait=keep,
                        on_update=list(si.on_update) if si.on_update else [],
                    )
                out.append(inst)
            if changed:
                bb.instructions = out


def _patched_drain_and_barrier(self, tick_clock, wait_clock):
    from concourse.vector_clock import ScopedClock

    nc = self.nc
    drain_inst = nc.sync.drain()
    wait_clock.add_sem_waits(
        drain_inst.ins, ScopedClock({None: tick_clock.global_clock})
    )
    nc.all_engine_barrier()
    assert self.sems is not None
    popped = nc._tile_sem_poison_stack.pop()
    assert popped is self._sem_poison
    nc.clear_and_free_semaphores(list(self.sems.allocated().values()))
    nc.all_engine_barrier()
    _split_waits(nc)


tile.TileContext._drain_and_barrier = _patched_drain_and_barrier

_NC_CACHE = None
RUN_KWARGS: dict = {}
LAST_RES = None


def _build():
    nc = bass.Bass("TRN2", target_bir_lowering=False, debug=False)

    anT_d = nc.dram_tensor("anT", [128, ANT_COLS], BF16, kind="ExternalInput").ap()
    bnT_d = nc.dram_tensor("bnT", [128, N], BF16, kind="ExternalInput").ap()

    acc11_d = nc.dram_tensor("acc11", [128, 8 * TILES], F32, kind="ExternalOutput").ap()
    acc22_d = nc.dram_tensor("acc22", [128, 8 * TILES], F32, kind="ExternalOutput").ap()
    acc12_d = nc.dram_tensor("acc12", [128, 16 * TILES], F32, kind="ExternalOutput").ap()
    band11_d = nc.dram_tensor("band11", [128, R], BF16, kind="ExternalOutput").ap()
    band22_d = nc.dram_tensor("band22", [128, R], BF16, kind="ExternalOutput").ap()
    ca11_d = nc.dram_tensor("ca11", [128, CA_COLS], BF16, kind="ExternalOutput").ap()
    ca22_d = nc.dram_tensor("ca22", [128, CA_COLS], BF16, kind="ExternalOutput").ap()
    ca12_d = nc.dram_tensor("ca12", [128, N], BF16, kind="ExternalOutput").ap()

    with tile.TileContext(nc) as tc:
        with tc.tile_pool(name="pers", bufs=1) as pers:
            anT = pers.tile([128, ANT_COLS], BF16, tag="anT")
            bnT = pers.tile([128, N], BF16, tag="bnT")
            ca12sb = pers.tile([128, POOL12_P * PAIR], BF16, tag="ca12sb")
            acc11 = pers.tile([128, 8 * TILES], F32, tag="acc11")
            acc22 = pers.tile([128, 8 * TILES], F32, tag="acc22")
            acc12 = pers.tile([128, 16 * TILES], F32, tag="acc12")

            # input DMAs, chunked so the first window can start early
            for c0 in range(0, ANT_COLS, 4096):
                c1 = min(c0 + 4096, ANT_COLS)
                nc.sync.dma_start(anT[:, c0:c1], anT_d[:, c0:c1])
            for c0 in range(0, N, 4096):
                nc.sync.dma_start(bnT[:, c0 : c0 + 4096], bnT_d[:, c0 : c0 + 4096])

            with (
                tc.tile_pool(name="mp", bufs=4, space="PSUM") as mp,
                tc.tile_pool(name="ep", bufs=1) as ep,
                tc.tile_pool(name="scr", bufs=2) as scr,
            ):
                mats = [
                    (anT, anT, acc11, acc11_d, ca11_d, band11_d, True),
                    (bnT, bnT, acc22, acc22_d, ca22_d, band22_d, True),
                    (anT, bnT, acc12, acc12_d, ca12_d, None, False),
                ]
                noff = [0, 0]  # Bresenham state: [sub-chunks seen, offloaded]
                for lhs, rhs, acc, acc_d, ca_d, band_d, sym in mats:
                    nch = 8 if sym else 16
                    for m in range(TILES):
                        lT = lhs[:, m * 128 : (m + 1) * 128]
                        base = m * 128 if sym else 0
                        E4 = None
                        for k in range(nch):
                            c0 = base + k * CHUNK
                            ps = mp.tile([128, CHUNK], F32, tag="mm")
                            for q in range(2):
                                nc.tensor.matmul(
                                    ps[:, q * 512 : (q + 1) * 512],
                                    lT,
                                    rhs[:, c0 + q * 512 : c0 + (q + 1) * 512],
                                )
                            # E tiles pack 8 sub-chunks; colacc reads 2048-wide
                            # pair slices (hw limit for accumulate DMAs)
                            if k % 8 == 0:
                                E4 = ep.tile([128, 8 * CHUNK], BF16, tag="E4", bufs=3)
                            ke = (k % 8) * CHUNK
                            E = E4[:, ke : ke + CHUNK]
                            Ei = E4[:, ke : ke + CHUNK].bitcast(I16)
                            slot = m * nch + k
                            noff[0] += 1
                            off = noff[0] * OFF_NUM // OFF_DEN > noff[1]
                            if off:
                                noff[1] += 1
                                nc.vector.tensor_scalar(
                                    Ei, ps[:], SCHRA_A, SCHRA_B, OP.mult, OP.add
                                )
                                # fast rowsum: identity tensor_scalar keeps DVE 4x
                                # mode (scalar-shaped accum doesn't break it)
                                pscr = scr.tile([128, CHUNK], BF16, tag="pscr")
                                nc.vector.tensor_scalar(
                                    pscr[:],
                                    E,
                                    1.0,
                                    None,
                                    OP.mult,
                                    OP.add,
                                    accum_out=acc[:, slot : slot + 1],
                                )
                            else:
                                nc.scalar.activation(
                                    E,
                                    ps[:],
                                    AF.Exp,
                                    scale=INV_TAU,
                                    accum_out=acc[:, slot : slot + 1],
                                )
                            if k % 2 == 0:
                                continue
                            # pair p = k//2 complete: emit column accumulation
                            p = k // 2
                            pe0 = (p % 4) * PAIR  # pair offset within E4
                            if sym:
                                # region D in [1,63]: cols [m*128+128, m*128+8192)
                                # -> ca idx [m*128, m*128+8064); cols >= prev
                                # tile end (ca idx >= m*128+7936) first-touched
                                pc0 = base + p * PAIR
                                lo = max(pc0, m * 128 + 128)
                                hi = pc0 + PAIR
                                e0 = pe0 + (lo - pc0)
                                a0 = lo - 128
                                a1 = hi - 128
                                new0 = 128 if m == 0 else m * 128 + 8064
                                if lo >= new0:
                                    nc.gpsimd.dma_start(
                                        ca_d[:, a0:a1], E4[:, e0 : pe0 + PAIR]
                                    )
                                elif hi <= new0:
                                    nc.gpsimd.dma_start(
                                        ca_d[:, a0:a1],
                                        E4[:, e0 : pe0 + PAIR],
                                        accum_op=OP.add,
                                    )
                                else:
                                    sp = pe0 + (new0 - pc0)
                                    nc.gpsimd.dma_start(
                                        ca_d[:, a0 : new0 - 128],
                                        E4[:, e0:sp],
                                        accum_op=OP.add,
                                    )
                                    nc.gpsimd.dma_start(
                                        ca_d[:, new0 - 128 : a1],
                                        E4[:, sp : pe0 + PAIR],
                                    )
                            elif p < POOL12_P:
                                ve = nc.vector if p < DVE12_P else nc.gpsimd
                                pc0 = p * PAIR
                                if m == 0:
                                    ve.tensor_copy(
                                        ca12sb[:, pc0 : pc0 + PAIR],
                                        E4[:, pe0 : pe0 + PAIR],
                                    )
                                else:
                                    ve.tensor_tensor(
                                        ca12sb[:, pc0 : pc0 + PAIR],
                                        E4[:, pe0 : pe0 + PAIR],
                                        ca12sb[:, pc0 : pc0 + PAIR],
                                        OP.add,
                                    )
                            else:
                                pc0 = p * PAIR
                                if m == 0:
                                    nc.gpsimd.dma_start(
                                        ca_d[:, pc0 : pc0 + PAIR],
                                        E4[:, pe0 : pe0 + PAIR],
                                    )
                                else:
                                    nc.gpsimd.dma_start(
                                        ca_d[:, pc0 : pc0 + PAIR],
                                        E4[:, pe0 : pe0 + PAIR],
                                        accum_op=OP.add,
                                    )
                    if sym:
                        # band pass: distance-64 tiles (m, m+64), rowsum-only,
                        # raw exps exported; host reduces.
                        for h in range(2):
                            ps = mp.tile([128, CHUNK], F32, tag="mm")
                            for j in range(8):
                                m = h * 8 + j
                                nc.tensor.matmul(
                                    ps[:, j * 128 : (j + 1) * 128],
                                    lhs[:, m * 128 : (m + 1) * 128],
                                    rhs[:, WIN + m * 128 : WIN + (m + 1) * 128],
                                )
                            Eb = ep.tile([128, CHUNK], BF16, tag="Eb", bufs=2)
                            nc.scalar.activation(Eb[:], ps[:], AF.Exp, scale=INV_TAU)
                            nc.sync.dma_start(
                                band_d[:, h * CHUNK : (h + 1) * CHUNK], Eb[:]
                            )
                    nc.sync.dma_start(acc_d[:, :], acc[:, :])
                    if not sym:
                        for c0 in range(0, POOL12_P * PAIR, PAIR):
                            nc.sync.dma_start(
                                ca_d[:, c0 : c0 + PAIR], ca12sb[:, c0 : c0 + PAIR]
                            )

    return nc


def _get_nc():
    global _NC_CACHE
    if _NC_CACHE is None:
        _NC_CACHE = _build()
    return _NC_CACHE


def _project(z, W1, b1, W2, b2):
    u = z @ W1 + b1
    h = np.where(u > 0, u, np.expm1(np.minimum(u, 0.0))) @ W2 + b2
    n = np.sqrt(np.sum(h * h, axis=1, keepdims=True))
    return h / np.maximum(n, 1e-12)


def kernel(z1, z2, W1, b1, W2, b2):
    global LAST_RES
    bf = ml_dtypes.bfloat16
    z1 = np.asarray(z1, dtype=np.float32)
    z2 = np.asarray(z2, dtype=np.float32)
    W1 = np.asarray(W1, dtype=np.float32)
    W2 = np.asarray(W2, dtype=np.float32)
    b1 = np.asarray(b1, dtype=np.float32)
    b2 = np.asarray(b2, dtype=np.float32)

    an = _project(z1, W1, b1, W2, b2)
    bn = _project(z2, W1, b1, W2, b2)
    anT_bf = np.ascontiguousarray(an.T).astype(bf)  # [128, N]
    bnT_bf = np.ascontiguousarray(bn.T).astype(bf)

    nc = _get_nc()
    in_maps = []
    for c in range(NCORES):
        a = np.roll(anT_bf, -c * R, axis=1)
        b = np.roll(bnT_bf, -c * R, axis=1)
        in_maps.append(
            {
                "anT": np.ascontiguousarray(a[:, :ANT_COLS]),
                "bnT": np.ascontiguousarray(b),
            }
        )
    res = run_bass_kernel_spmd(nc, in_maps, list(range(NCORES)), **RUN_KWARGS)
    LAST_RES = res

    e2 = np.exp(np.float64(INV_TAU))
    den1 = np.zeros(N, np.float64)
    den2 = np.zeros(N, np.float64)
    idx_ca = None
    for c in range(NCORES):
        r = res.results[c]
        own = slice(c * R, (c + 1) * R)
        # windowed row sums: acc[p, m*nch+k] for row m*128+p
        a11 = r["acc11"].astype(np.float64).reshape(128, TILES, 8)
        a22 = r["acc22"].astype(np.float64).reshape(128, TILES, 8)
        a12 = r["acc12"].astype(np.float64).reshape(128, TILES, 16)
        rs11 = a11.sum(axis=2).T.reshape(R)  # [m,p] -> row m*128+p
        rs22 = a22.sum(axis=2).T.reshape(R)
        rs12 = a12.sum(axis=2).T.reshape(R)
        # band: E[p, m*128+j] = exp tile (m, m+64) -> row m*128+p sums over j
        b11 = r["band11"].astype(np.float64).reshape(128, TILES, 128)
        b22 = r["band22"].astype(np.float64).reshape(128, TILES, 128)
        rs11 += b11.sum(axis=2).T.reshape(R)
        rs22 += b22.sum(axis=2).T.reshape(R)
        den1[own] += rs11 + rs12
        den2[own] += rs22
        # column accumulators: partition-sum then roll to global rows
        cs11 = r["ca11"].astype(np.float64).sum(axis=0)  # local col j+128
        cs22 = r["ca22"].astype(np.float64).sum(axis=0)
        cs12 = r["ca12"].astype(np.float64).sum(axis=0)  # local col j
        if idx_ca is None:
            idx_ca = np.arange(CA_COLS)
        den1[(c * R + 128 + idx_ca) % N] += cs11
        den2[(c * R + 128 + idx_ca) % N] += cs22
        den2[(c * R + np.arange(N)) % N] += cs12
    den1 -= e2
    den2 -= e2

    lognum = 2.0 * np.sum(an.astype(np.float64) * bn.astype(np.float64), axis=1)
    loss = np.mean(0.5 * (np.log(den1) + np.log(den2)) - lognum)
    return np.array(loss, dtype=np.float32)
